# revision 14
# baseline (speedup 1.0000x reference)
"""Trainium2 Bass kernel for nn_ConvAE: scores=relu(x@W.T); idx=argmax_P(scores); out[b,idx[b,c],:]+=W[c].

Sharding: data-parallel over batch B=8 across 8 cores (full W replica per core).
Per core: x_b [4096, 256], W [1024, 256] -> out_b [4096, 256].

Pipeline per core:
  1. PE transposes W -> WT [d, C] and x_b -> xT [d, P] (identity matmuls).
  2. PE computes scoresT[c, p] = sum_d WT[d,c] * xT[d,p] in PSUM (fp32).
     relu is skipped: argmax(relu(s)) == argmax(s) whenever max(s) > 0
     (P(all 4096 scores <= 0) ~ 2^-4096).
  3. ScalarE evicts scoresT to SBUF; DVE finds per-512-chunk top-8 (InstMax),
     global max, winning chunk; GPSIMD indirect_copy gathers each channel's
     winning chunk; DVE InstMaxIndex gives the first-occurrence argmax
     (matches jnp.argmax tie semantics).
  4. Collision handling: E[c,c'] = (idx[c]==idx[c']); combined = E @ W sums
     W-rows of channels that share a target patch. Duplicate scatter targets
     then carry identical payloads, so racy DMA writes are benign.
  5. indirect_dma_start scatters combined rows to out[idx[c], :]. Rows never
     hit stay zero: ExternalOutput buffers are pre-zeroed by the runtime.
"""

import os
import sys

import numpy as np

for _p in ("/opt/trn_rl_repo", "/root/.axon_site/_ro/trn_rl_repo"):
    if os.path.isdir(_p) and _p not in sys.path:
        sys.path.insert(0, _p)

import concourse.bass as bass  # noqa: E402
import concourse.mybir as mybir  # noqa: E402
import concourse.tile as tile  # noqa: E402
from concourse import bacc  # noqa: E402
from concourse.bass import IndirectOffsetOnAxis  # noqa: E402
from concourse.bass_utils import run_bass_kernel_spmd  # noqa: E402
from concourse.masks import make_identity  # noqa: E402

F32 = mybir.dt.float32
I32 = mybir.dt.int32
U32 = mybir.dt.uint32
U16 = mybir.dt.uint16

B, P, D, C = 8, 4096, 256, 1024
PT = 128          # partition tile
NCT = C // PT     # 8 channel tiles
PCH = 512         # p-chunk width for matmul / max
NPC = P // PCH    # 8 p chunks
NDH = D // PT     # 2 contraction halves

_NC_CACHE = {}


def _build_nc():
    nc = bacc.Bacc("TRN2", target_bir_lowering=False, debug=False, num_devices=B)
    x_d = nc.dram_tensor("x", [P, D], F32, kind="ExternalInput")
    w_d = nc.dram_tensor("w", [C, D], F32, kind="ExternalInput")
    o_d = nc.dram_tensor("o", [P, D], F32, kind="ExternalOutput")
    alu = mybir.AluOpType

    with tile.TileContext(nc) as tc:
        with (
            tc.tile_pool(name="sb", bufs=1) as sb,
            tc.tile_pool(name="sbs", bufs=2) as sbs,
            tc.tile_pool(name="pp", bufs=2, space="PSUM") as pp,
        ):
            ident = sb.tile([PT, PT], F32)
            make_identity(nc, ident[:])



            # ---- load W wrapped [p, j, d]: row j*128+p ----
            w_sb = sb.tile([PT, NCT, D], F32)
            nc.sync.dma_start(w_sb[:], w_d[:].rearrange("(j p) d -> p j d", p=PT))

            # ---- WT [d-half, c] ----
            wt_sb = sb.tile([PT, NDH, C], F32)
            for h in range(NDH):
                for g in range(2):
                    pt = pp.tile([PT, 512], F32, tag="pt")
                    for k in range(4):
                        j = 4 * g + k
                        nc.tensor.transpose(
                            pt[:, 128 * k:128 * (k + 1)],
                            w_sb[:, j, 128 * h:128 * (h + 1)],
                            ident[:],
                        )
                    nc.scalar.copy(wt_sb[:, h, 512 * g:512 * (g + 1)], pt[:])

            # ---- load x chunks, build xT [d-half, p] ----
            xt_sb = sb.tile([PT, NDH, P], F32)
            x_view = x_d[:].rearrange("(c s p) d -> c p s d", s=4, p=PT)
            for pc in range(NPC):
                x_sb = sbs.tile([PT, 4, D], F32, tag="x", bufs=3)
                nc.sync.dma_start(x_sb[:], x_view[pc])
                for h in range(NDH):
                    pxt = pp.tile([PT, 512], F32, tag="pt")
                    for s in range(4):
                        nc.tensor.transpose(
                            pxt[:, 128 * s:128 * (s + 1)],
                            x_sb[:, s, 128 * h:128 * (h + 1)],
                            ident[:],
                        )
                    nc.scalar.copy(xt_sb[:, h, PCH * pc:PCH * (pc + 1)], pxt[:])

            # ---- main: scoresT per channel-tile; argmax over p ----
            idx_f = sb.tile([PT, NCT], F32)
            for ct in range(NCT):
                scores = sbs.tile([PT, P], F32, tag="scores")
                for g in range(4):  # 2 p-chunks per psum tile
                    ps = pp.tile([PT, 2 * PCH], F32, tag="ps")
                    for q in range(2):
                        pc = 2 * g + q
                        for h in range(NDH):
                            nc.tensor.matmul(
                                ps[:, PCH * q:PCH * (q + 1)],
                                lhsT=wt_sb[:, h, PT * ct:PT * (ct + 1)],
                                rhs=xt_sb[:, h, PCH * pc:PCH * (pc + 1)],
                                start=(h == 0),
                                stop=(h == NDH - 1),
                            )
                    nc.scalar.copy(scores[:, 1024 * g:1024 * (g + 1)], ps[:])
                gmax8 = sbs.tile([PT, 8], F32, tag="gmax8")
                nc.vector.max(gmax8[:], scores[:])
                pidx = sbs.tile([PT, 8], U32, tag="pidx8")
                nc.vector.max_index(pidx[:], gmax8[:], scores[:])
                nc.vector.tensor_copy(idx_f[:, ct:ct + 1], pidx[:, 0:1])

            # ---- idxT[p, c'] = idx[c'] ----
            idxT = sb.tile([PT, C], F32)
            for ct in range(NCT):
                pidx = pp.tile([PT, PT], F32, tag="pt")
                nc.tensor.transpose(
                    pidx[:], idx_f[:, ct:ct + 1].to_broadcast([PT, PT]), ident[:]
                )
                nc.scalar.copy(idxT[:, PT * ct:PT * (ct + 1)], pidx[:])

            # ---- E[c, c'] = (idx[c] == idx[c']) ----
            e_sb = sb.tile([PT, NCT, C], F32, tag="big", bufs=1)
            for ct in range(NCT):
                nc.vector.tensor_scalar(
                    e_sb[:, ct, :], idxT[:], idx_f[:, ct:ct + 1], None, op0=alu.is_equal
                )

            # ---- combinedT[d, 1+c] = sum_c' W[c', d] * E[c', c]; col 0 = zeros ----
            combT = sb.tile([PT, NDH, 1 + C], F32)
            for h in range(NDH):
                nc.vector.memset(combT[:, h, 0:1], 0.0)
                for ch in range(2):
                    pcm = pp.tile([PT, 512], F32, tag="pt")
                    for j in range(NCT):
                        nc.tensor.matmul(
                            pcm[:],
                            lhsT=w_sb[:, j, PT * h:PT * (h + 1)],
                            rhs=e_sb[:, j, 512 * ch:512 * (ch + 1)],
                            start=(j == 0),
                            stop=(j == NCT - 1),
                        )
                    nc.scalar.copy(combT[:, h, 1 + 512 * ch:1 + 512 * (ch + 1)], pcm[:])

            # ---- inverse table, wrapped for indirect_copy: partition r holds
            # inv[i] for i % 16 == r % 16 at column i // 16; value = 1+c or 0 ----
            p16 = sb.tile([PT, 1], I32)
            nc.gpsimd.iota(p16[:], [[0, 1]], base=0, channel_multiplier=1)
            nc.vector.tensor_scalar(p16[:], p16[:], 15, None, op0=alu.bitwise_and)
            p16f = sb.tile([PT, 1], F32)
            nc.vector.tensor_copy(p16f[:], p16[:])
            iota_c1 = sb.tile([PT, C], U16)
            nc.gpsimd.iota(iota_c1[:], [[1, C]], base=1, channel_multiplier=0)

            idxT_i = sb.tile([PT, C], I32)
            nc.vector.tensor_copy(idxT_i[:], idxT[:])
            rmod_i = sb.tile([PT, C], I32)
            nc.vector.tensor_scalar(rmod_i[:], idxT_i[:], 15, None, op0=alu.bitwise_and)
            rmod = sb.tile([PT, C], F32)
            nc.vector.tensor_copy(rmod[:], rmod_i[:])
            mask = sb.tile([PT, C], F32)
            nc.vector.tensor_scalar(mask[:], rmod[:], p16f[:], None, op0=alu.is_equal)
            col1 = sb.tile([PT, C], F32)  # idxT // 16 + 1
            nc.vector.tensor_tensor(col1[:], idxT[:], rmod[:], op=alu.subtract)
            nc.vector.tensor_scalar(col1[:], col1[:], 1.0 / 16.0, 1.0, op0=alu.mult, op1=alu.add)
            sc_idx_f = sb.tile([PT, C], F32)  # mask*col1 - 1 (-1 = ignore)
            nc.vector.tensor_tensor(sc_idx_f[:], mask[:], col1[:], op=alu.mult)
            nc.vector.tensor_scalar(sc_idx_f[:], sc_idx_f[:], 1.0, None, op0=alu.subtract)
            sc_idx = sb.tile([PT, C], mybir.dt.int16)
            nc.vector.tensor_copy(sc_idx[:], sc_idx_f[:])

            inv_w = sb.tile([PT, P // 16], U16)
            nc.gpsimd.local_scatter(
                out_ap=inv_w[:],
                data_ap=iota_c1[:],
                idxs_ap=sc_idx[:],
                channels=PT,
                num_elems=P // 16,
                num_idxs=C,
            )

            # ---- outT[d, p] = combT[d, inv[p]]; transpose back; store ----
            outN = sb.tile([PT, NDH, P // PT, PT], F32, tag="big", bufs=1)
            for h in range(NDH):
                outT = sbs.tile([PT, P], F32, tag="outT", bufs=1)
                for k in range(4):  # ISA: <=1024 dst elems per indirect_copy
                    nc.gpsimd.indirect_copy(
                        outT[:, 1024 * k:1024 * (k + 1)],
                        data=combT[:, h, :],
                        idxs=inv_w[:, 64 * k:64 * (k + 1)],
                        i_know_ap_gather_is_preferred=True,
                    )
                for g in range(8):
                    pot = pp.tile([PT, 512], F32, tag="pt")
                    for s in range(4):
                        t = 4 * g + s
                        nc.tensor.transpose(
                            pot[:, 128 * s:128 * (s + 1)],
                            outT[:, PT * t:PT * (t + 1)],
                            ident[:],
                        )
                    nc.scalar.copy(
                        outN[:, h, 4 * g:4 * (g + 1), :],
                        pot[:].rearrange("p (s dd) -> p s dd", dd=PT),
                    )
            o_view = o_d[:].rearrange("(q t p) (h dd) -> h q p t dd", q=4, p=PT, dd=PT)
            for h in range(NDH):
                for q in range(4):
                    nc.sync.dma_start(o_view[h][q], outN[:, h, 8 * q:8 * (q + 1), :])

    nc.compile()
    return nc


def _get_nc():
    if "nc" not in _NC_CACHE:
        _NC_CACHE["nc"] = _build_nc()
    return _NC_CACHE["nc"]


def kernel(x: np.ndarray, W: np.ndarray) -> np.ndarray:
    x = np.ascontiguousarray(x, dtype=np.float32)
    W = np.ascontiguousarray(W, dtype=np.float32)
    assert x.shape == (B, P, D) and W.shape == (C, D)
    nc = _get_nc()
    in_maps = [{"x": x[b], "w": W} for b in range(B)]
    res = run_bass_kernel_spmd(nc, in_maps, core_ids=list(range(B)))
    out = np.stack([res.results[b]["o"] for b in range(B)], axis=0)
    return out.astype(np.float32)


if __name__ == "__main__":
    rng = np.random.default_rng(0)
    x = rng.standard_normal((B, P, D), dtype=np.float32)
    W = (rng.standard_normal((C, D), dtype=np.float32) * 0.001).astype(np.float32)
    out = kernel(x=x, W=W)
    print(out.shape, out.dtype, float(np.abs(out).sum()))


# revision 16
# speedup vs baseline: 1.1309x; 1.1309x over previous
"""Trainium2 Bass kernel for nn_ConvAE: scores=relu(x@W.T); idx=argmax_P(scores); out[b,idx[b,c],:]+=W[c].

Sharding: data-parallel over batch B=8 across 8 cores (full W replica per core).
Per core: x_b [4096, 256], W [1024, 256] -> out_b [4096, 256].

Pipeline per core:
  1. PE transposes W -> WT [d, C] and x_b -> xT [d, P] (identity matmuls).
  2. PE computes scoresT[c, p] = sum_d WT[d,c] * xT[d,p] in PSUM (fp32).
     relu is skipped: argmax(relu(s)) == argmax(s) whenever max(s) > 0
     (P(all 4096 scores <= 0) ~ 2^-4096).
  3. ScalarE evicts scoresT to SBUF; DVE finds per-512-chunk top-8 (InstMax),
     global max, winning chunk; GPSIMD indirect_copy gathers each channel's
     winning chunk; DVE InstMaxIndex gives the first-occurrence argmax
     (matches jnp.argmax tie semantics).
  4. Collision handling: E[c,c'] = (idx[c]==idx[c']); combined = E @ W sums
     W-rows of channels that share a target patch. Duplicate scatter targets
     then carry identical payloads, so racy DMA writes are benign.
  5. indirect_dma_start scatters combined rows to out[idx[c], :]. Rows never
     hit stay zero: ExternalOutput buffers are pre-zeroed by the runtime.
"""

import os
import sys

import numpy as np

for _p in ("/opt/trn_rl_repo", "/root/.axon_site/_ro/trn_rl_repo"):
    if os.path.isdir(_p) and _p not in sys.path:
        sys.path.insert(0, _p)

import concourse.bass as bass  # noqa: E402
import concourse.mybir as mybir  # noqa: E402
import concourse.tile as tile  # noqa: E402
from concourse import bacc  # noqa: E402
from concourse.bass import IndirectOffsetOnAxis  # noqa: E402
from concourse.bass_utils import run_bass_kernel_spmd  # noqa: E402
from concourse.masks import make_identity  # noqa: E402

F32 = mybir.dt.float32
I32 = mybir.dt.int32
U32 = mybir.dt.uint32
U16 = mybir.dt.uint16
F32R = mybir.dt.float32r

B, P, D, C = 8, 4096, 256, 1024
PT = 128          # partition tile
NCT = C // PT     # 8 channel tiles
PCH = 512         # p-chunk width for matmul / max
NPC = P // PCH    # 8 p chunks
NDH = D // PT     # 2 contraction halves

_NC_CACHE = {}


def _build_nc():
    nc = bacc.Bacc("TRN2", target_bir_lowering=False, debug=False, num_devices=B)
    x_d = nc.dram_tensor("x", [P, D], F32, kind="ExternalInput")
    w_d = nc.dram_tensor("w", [C, D], F32, kind="ExternalInput")
    o_d = nc.dram_tensor("o", [P, D], F32, kind="ExternalOutput")
    alu = mybir.AluOpType

    with tile.TileContext(nc) as tc:
        with (
            tc.tile_pool(name="sb", bufs=1) as sb,
            tc.tile_pool(name="sbs", bufs=2) as sbs,
            tc.tile_pool(name="pp", bufs=2, space="PSUM") as pp,
        ):
            ident = sb.tile([PT, PT], F32)
            make_identity(nc, ident[:])



            # ---- load W wrapped [p, j, d]: row j*128+p ----
            w_sb = sb.tile([PT, NCT, D], F32)
            nc.sync.dma_start(w_sb[:], w_d[:].rearrange("(j p) d -> p j d", p=PT))

            # ---- WT [d-half, c] ----
            wt_sb = sb.tile([PT, NDH, C], F32R)
            for h in range(NDH):
                for g in range(2):
                    pt = pp.tile([PT, 512], F32, tag="pt")
                    for k in range(4):
                        j = 4 * g + k
                        nc.tensor.transpose(
                            pt[:, 128 * k:128 * (k + 1)],
                            w_sb[:, j, 128 * h:128 * (h + 1)],
                            ident[:],
                        )
                    nc.scalar.copy(wt_sb[:, h, 512 * g:512 * (g + 1)], pt[:])

            # ---- load x chunks, build xT [d-half, p] ----
            xt_sb = sb.tile([PT, NDH, P], F32R)
            x_view = x_d[:].rearrange("(c s p) d -> c p s d", s=4, p=PT)
            for pc in range(NPC):
                x_sb = sbs.tile([PT, 4, D], F32, tag="x", bufs=3)
                nc.sync.dma_start(x_sb[:], x_view[pc])
                for h in range(NDH):
                    pxt = pp.tile([PT, 512], F32, tag="pt")
                    for s in range(4):
                        nc.tensor.transpose(
                            pxt[:, 128 * s:128 * (s + 1)],
                            x_sb[:, s, 128 * h:128 * (h + 1)],
                            ident[:],
                        )
                    nc.scalar.copy(xt_sb[:, h, PCH * pc:PCH * (pc + 1)], pxt[:])

            # ---- main: scoresT per channel-tile; argmax over p ----
            idx_f = sb.tile([PT, NCT], F32)
            for ct in range(NCT):
                scores = sbs.tile([PT, P], F32, tag="scores")
                for g in range(4):  # 2 p-chunks per psum tile
                    ps = pp.tile([PT, 2 * PCH], F32, tag="ps")
                    for q in range(2):
                        pc = 2 * g + q
                        for h in range(NDH):
                            nc.tensor.matmul(
                                ps[:, PCH * q:PCH * (q + 1)],
                                lhsT=wt_sb[:, h, PT * ct:PT * (ct + 1)],
                                rhs=xt_sb[:, h, PCH * pc:PCH * (pc + 1)],
                                start=(h == 0),
                                stop=(h == NDH - 1),
                            )
                    nc.scalar.copy(scores[:, 1024 * g:1024 * (g + 1)], ps[:])
                gmax8 = sbs.tile([PT, 8], F32, tag="gmax8")
                nc.vector.max(gmax8[:], scores[:])
                pidx = sbs.tile([PT, 8], U32, tag="pidx8")
                nc.vector.max_index(pidx[:], gmax8[:], scores[:])
                nc.vector.tensor_copy(idx_f[:, ct:ct + 1], pidx[:, 0:1])

            # ---- idxT[p, c'] = idx[c'] ----
            idxT = sb.tile([PT, C], F32)
            for ct in range(NCT):
                pidx = pp.tile([PT, PT], F32, tag="pt")
                nc.tensor.transpose(
                    pidx[:], idx_f[:, ct:ct + 1].to_broadcast([PT, PT]), ident[:]
                )
                nc.scalar.copy(idxT[:, PT * ct:PT * (ct + 1)], pidx[:])

            # ---- E[c, c'] = (idx[c] == idx[c']) ----
            e_sb = sb.tile([PT, NCT, C], F32R, tag="big", bufs=1)
            for ct in range(NCT):
                nc.vector.tensor_scalar(
                    e_sb[:, ct, :], idxT[:], idx_f[:, ct:ct + 1], None, op0=alu.is_equal
                )

            # ---- combinedT[d, 1+c] = sum_c' W[c', d] * E[c', c]; col 0 = zeros ----
            combT = sb.tile([PT, NDH, 1 + C], F32)
            w_r = sb.tile([PT, NCT, D], F32R)
            nc.scalar.copy(w_r[:], w_sb[:])
            for h in range(NDH):
                nc.vector.memset(combT[:, h, 0:1], 0.0)
                for ch in range(2):
                    pcm = pp.tile([PT, 512], F32, tag="pt")
                    for j in range(NCT):
                        nc.tensor.matmul(
                            pcm[:],
                            lhsT=w_r[:, j, PT * h:PT * (h + 1)],
                            rhs=e_sb[:, j, 512 * ch:512 * (ch + 1)],
                            start=(j == 0),
                            stop=(j == NCT - 1),
                        )
                    nc.scalar.copy(combT[:, h, 1 + 512 * ch:1 + 512 * (ch + 1)], pcm[:])

            # ---- inverse table, wrapped for indirect_copy: partition r holds
            # inv[i] for i % 16 == r % 16 at column i // 16; value = 1+c or 0 ----
            p16 = sb.tile([PT, 1], I32)
            nc.gpsimd.iota(p16[:], [[0, 1]], base=0, channel_multiplier=1)
            nc.vector.tensor_scalar(p16[:], p16[:], 15, None, op0=alu.bitwise_and)
            p16f = sb.tile([PT, 1], F32)
            nc.vector.tensor_copy(p16f[:], p16[:])
            iota_c1 = sb.tile([PT, C], U16)
            nc.gpsimd.iota(iota_c1[:], [[1, C]], base=1, channel_multiplier=0)

            idxT_i = sb.tile([PT, C], I32)
            nc.vector.tensor_copy(idxT_i[:], idxT[:])
            rmod_i = sb.tile([PT, C], I32)
            nc.vector.tensor_scalar(rmod_i[:], idxT_i[:], 15, None, op0=alu.bitwise_and)
            rmod = sb.tile([PT, C], F32)
            nc.vector.tensor_copy(rmod[:], rmod_i[:])
            mask = sb.tile([PT, C], F32)
            nc.vector.tensor_scalar(mask[:], rmod[:], p16f[:], None, op0=alu.is_equal)
            col1 = sb.tile([PT, C], F32)  # idxT // 16 + 1
            nc.vector.tensor_tensor(col1[:], idxT[:], rmod[:], op=alu.subtract)
            nc.vector.tensor_scalar(col1[:], col1[:], 1.0 / 16.0, 1.0, op0=alu.mult, op1=alu.add)
            sc_idx_f = sb.tile([PT, C], F32)  # mask*col1 - 1 (-1 = ignore)
            nc.vector.tensor_tensor(sc_idx_f[:], mask[:], col1[:], op=alu.mult)
            nc.vector.tensor_scalar(sc_idx_f[:], sc_idx_f[:], 1.0, None, op0=alu.subtract)
            sc_idx = sb.tile([PT, C], mybir.dt.int16)
            nc.vector.tensor_copy(sc_idx[:], sc_idx_f[:])

            inv_w = sb.tile([PT, P // 16], U16)
            nc.gpsimd.local_scatter(
                out_ap=inv_w[:],
                data_ap=iota_c1[:],
                idxs_ap=sc_idx[:],
                channels=PT,
                num_elems=P // 16,
                num_idxs=C,
            )

            # ---- outT[d, p] = combT[d, inv[p]]; transpose back; store ----
            outN = sb.tile([PT, NDH, P // PT, PT], F32, tag="big", bufs=1)
            for h in range(NDH):
                outT = sbs.tile([PT, P], F32, tag="outT", bufs=1)
                for k in range(4):  # ISA: <=1024 dst elems per indirect_copy
                    nc.gpsimd.indirect_copy(
                        outT[:, 1024 * k:1024 * (k + 1)],
                        data=combT[:, h, :],
                        idxs=inv_w[:, 64 * k:64 * (k + 1)],
                        i_know_ap_gather_is_preferred=True,
                    )
                for g in range(8):
                    pot = pp.tile([PT, 512], F32, tag="pt")
                    for s in range(4):
                        t = 4 * g + s
                        nc.tensor.transpose(
                            pot[:, 128 * s:128 * (s + 1)],
                            outT[:, PT * t:PT * (t + 1)],
                            ident[:],
                        )
                    nc.scalar.copy(
                        outN[:, h, 4 * g:4 * (g + 1), :],
                        pot[:].rearrange("p (s dd) -> p s dd", dd=PT),
                    )
            o_view = o_d[:].rearrange("(q t p) (h dd) -> h q p t dd", q=4, p=PT, dd=PT)
            for h in range(NDH):
                for q in range(4):
                    nc.sync.dma_start(o_view[h][q], outN[:, h, 8 * q:8 * (q + 1), :])

    nc.compile()
    return nc


def _get_nc():
    if "nc" not in _NC_CACHE:
        _NC_CACHE["nc"] = _build_nc()
    return _NC_CACHE["nc"]


def kernel(x: np.ndarray, W: np.ndarray) -> np.ndarray:
    x = np.ascontiguousarray(x, dtype=np.float32)
    W = np.ascontiguousarray(W, dtype=np.float32)
    assert x.shape == (B, P, D) and W.shape == (C, D)
    nc = _get_nc()
    in_maps = [{"x": x[b], "w": W} for b in range(B)]
    res = run_bass_kernel_spmd(nc, in_maps, core_ids=list(range(B)))
    out = np.stack([res.results[b]["o"] for b in range(B)], axis=0)
    return out.astype(np.float32)


if __name__ == "__main__":
    rng = np.random.default_rng(0)
    x = rng.standard_normal((B, P, D), dtype=np.float32)
    W = (rng.standard_normal((C, D), dtype=np.float32) * 0.001).astype(np.float32)
    out = kernel(x=x, W=W)
    print(out.shape, out.dtype, float(np.abs(out).sum()))


# revision 17
# speedup vs baseline: 1.1318x; 1.0008x over previous
"""Trainium2 Bass kernel for nn_ConvAE: scores=relu(x@W.T); idx=argmax_P(scores); out[b,idx[b,c],:]+=W[c].

Sharding: data-parallel over batch B=8 across 8 cores (full W replica per core).
Per core: x_b [4096, 256], W [1024, 256] -> out_b [4096, 256].

Pipeline per core:
  1. PE transposes W -> WT [d, C] and x_b -> xT [d, P] (identity matmuls).
  2. PE computes scoresT[c, p] = sum_d WT[d,c] * xT[d,p] in PSUM (fp32).
     relu is skipped: argmax(relu(s)) == argmax(s) whenever max(s) > 0
     (P(all 4096 scores <= 0) ~ 2^-4096).
  3. ScalarE evicts scoresT to SBUF; DVE finds per-512-chunk top-8 (InstMax),
     global max, winning chunk; GPSIMD indirect_copy gathers each channel's
     winning chunk; DVE InstMaxIndex gives the first-occurrence argmax
     (matches jnp.argmax tie semantics).
  4. Collision handling: E[c,c'] = (idx[c]==idx[c']); combined = E @ W sums
     W-rows of channels that share a target patch. Duplicate scatter targets
     then carry identical payloads, so racy DMA writes are benign.
  5. indirect_dma_start scatters combined rows to out[idx[c], :]. Rows never
     hit stay zero: ExternalOutput buffers are pre-zeroed by the runtime.
"""

import os
import sys

import numpy as np

for _p in ("/opt/trn_rl_repo", "/root/.axon_site/_ro/trn_rl_repo"):
    if os.path.isdir(_p) and _p not in sys.path:
        sys.path.insert(0, _p)

import concourse.bass as bass  # noqa: E402
import concourse.mybir as mybir  # noqa: E402
import concourse.tile as tile  # noqa: E402
from concourse import bacc  # noqa: E402
from concourse.bass import IndirectOffsetOnAxis  # noqa: E402
from concourse.bass_utils import run_bass_kernel_spmd  # noqa: E402
from concourse.masks import make_identity  # noqa: E402

F32 = mybir.dt.float32
I32 = mybir.dt.int32
U32 = mybir.dt.uint32
U16 = mybir.dt.uint16
F32R = mybir.dt.float32r

B, P, D, C = 8, 4096, 256, 1024
PT = 128          # partition tile
NCT = C // PT     # 8 channel tiles
PCH = 512         # p-chunk width for matmul / max
NPC = P // PCH    # 8 p chunks
NDH = D // PT     # 2 contraction halves

_NC_CACHE = {}


def _build_nc():
    nc = bacc.Bacc("TRN2", target_bir_lowering=False, debug=False, num_devices=B)
    x_d = nc.dram_tensor("x", [P, D], F32, kind="ExternalInput")
    w_d = nc.dram_tensor("w", [C, D], F32, kind="ExternalInput")
    o_d = nc.dram_tensor("o", [P, D], F32, kind="ExternalOutput")
    alu = mybir.AluOpType

    with tile.TileContext(nc) as tc:
        with (
            tc.tile_pool(name="sb", bufs=1) as sb,
            tc.tile_pool(name="sbs", bufs=2) as sbs,
            tc.tile_pool(name="pp", bufs=2, space="PSUM") as pp,
        ):
            ident = sb.tile([PT, PT], F32)
            make_identity(nc, ident[:])



            # ---- load W wrapped [p, j, d]: row j*128+p ----
            w_sb = sb.tile([PT, NCT, D], F32)
            nc.sync.dma_start(w_sb[:], w_d[:].rearrange("(j p) d -> p j d", p=PT))

            # ---- WT [d-half, c] ----
            wt_sb = sb.tile([PT, NDH, C], F32R)
            for h in range(NDH):
                for g in range(2):
                    pt = pp.tile([PT, 512], F32, tag="pt")
                    for k in range(4):
                        j = 4 * g + k
                        nc.tensor.transpose(
                            pt[:, 128 * k:128 * (k + 1)],
                            w_sb[:, j, 128 * h:128 * (h + 1)],
                            ident[:],
                        )
                    nc.scalar.copy(wt_sb[:, h, 512 * g:512 * (g + 1)], pt[:])

            # ---- load x chunks, build xT [d-half, p] ----
            xt_sb = sb.tile([PT, NDH, P], F32R)
            x_view = x_d[:].rearrange("(c s p) d -> c p s d", s=4, p=PT)
            for pc in range(NPC):
                x_sb = sbs.tile([PT, 4, D], F32, tag="x", bufs=3)
                nc.sync.dma_start(x_sb[:], x_view[pc])
                for h in range(NDH):
                    pxt = pp.tile([PT, 512], F32, tag="pt")
                    for s in range(4):
                        nc.tensor.transpose(
                            pxt[:, 128 * s:128 * (s + 1)],
                            x_sb[:, s, 128 * h:128 * (h + 1)],
                            ident[:],
                        )
                    nc.scalar.copy(xt_sb[:, h, PCH * pc:PCH * (pc + 1)], pxt[:])

            # ---- main: scoresT per channel-tile; argmax over p ----
            idx_f = sb.tile([PT, NCT], F32)
            for ct in range(NCT):
                scores = sbs.tile([PT, P], F32, tag="scores")
                for g in range(4):  # 2 p-chunks per psum tile
                    ps = pp.tile([PT, 2 * PCH], F32, tag="ps")
                    for q in range(2):
                        pc = 2 * g + q
                        for h in range(NDH):
                            nc.tensor.matmul(
                                ps[:, PCH * q:PCH * (q + 1)],
                                lhsT=wt_sb[:, h, PT * ct:PT * (ct + 1)],
                                rhs=xt_sb[:, h, PCH * pc:PCH * (pc + 1)],
                                start=(h == 0),
                                stop=(h == NDH - 1),
                            )
                    nc.scalar.copy(scores[:, 1024 * g:1024 * (g + 1)], ps[:])
                gmax8 = sbs.tile([PT, 8], F32, tag="gmax8")
                nc.vector.max(gmax8[:], scores[:])
                pidx = sbs.tile([PT, 8], U32, tag="pidx8")
                nc.vector.max_index(pidx[:], gmax8[:], scores[:])
                nc.vector.tensor_copy(idx_f[:, ct:ct + 1], pidx[:, 0:1])

            # ---- idxT[p, c'] = idx[c'] ----
            idxT = sb.tile([PT, C], F32)
            for ct in range(NCT):
                pidx = pp.tile([PT, PT], F32, tag="pt")
                nc.tensor.transpose(
                    pidx[:], idx_f[:, ct:ct + 1].to_broadcast([PT, PT]), ident[:]
                )
                nc.scalar.copy(idxT[:, PT * ct:PT * (ct + 1)], pidx[:])

            # ---- E[c, c'] = (idx[c] == idx[c']) ----
            e_sb = sb.tile([PT, NCT, C], F32R, tag="big", bufs=1)
            for ct in range(NCT):
                nc.vector.tensor_scalar(
                    e_sb[:, ct, :], idxT[:], idx_f[:, ct:ct + 1], None, op0=alu.is_equal
                )

            # ---- combinedT[d, 1+c] = sum_c' W[c', d] * E[c', c]; col 0 = zeros ----
            combT = sb.tile([PT, NDH, 1 + C], F32)
            # W = w_r + w_l, both fp32r-rounded -> combined is fp32-exact (~2^-26)
            w_r = sb.tile([PT, NCT, D], F32R)
            nc.scalar.copy(w_r[:], w_sb[:])
            w_l = sb.tile([PT, NCT, D], F32R)
            nc.vector.tensor_tensor(w_l[:], w_sb[:], w_r[:].bitcast(F32), op=alu.subtract)
            for h in range(NDH):
                nc.vector.memset(combT[:, h, 0:1], 0.0)
                for ch in range(2):
                    pcm = pp.tile([PT, 512], F32, tag="pt")
                    for j in range(NCT):
                        for wpart in (w_r, w_l):
                            nc.tensor.matmul(
                                pcm[:],
                                lhsT=wpart[:, j, PT * h:PT * (h + 1)],
                                rhs=e_sb[:, j, 512 * ch:512 * (ch + 1)],
                                start=(j == 0 and wpart is w_r),
                                stop=(j == NCT - 1 and wpart is w_l),
                            )
                    nc.scalar.copy(combT[:, h, 1 + 512 * ch:1 + 512 * (ch + 1)], pcm[:])

            # ---- inverse table, wrapped for indirect_copy: partition r holds
            # inv[i] for i % 16 == r % 16 at column i // 16; value = 1+c or 0 ----
            p16 = sb.tile([PT, 1], I32)
            nc.gpsimd.iota(p16[:], [[0, 1]], base=0, channel_multiplier=1)
            nc.vector.tensor_scalar(p16[:], p16[:], 15, None, op0=alu.bitwise_and)
            p16f = sb.tile([PT, 1], F32)
            nc.vector.tensor_copy(p16f[:], p16[:])
            iota_c1 = sb.tile([PT, C], U16)
            nc.gpsimd.iota(iota_c1[:], [[1, C]], base=1, channel_multiplier=0)

            idxT_i = sb.tile([PT, C], I32)
            nc.vector.tensor_copy(idxT_i[:], idxT[:])
            rmod_i = sb.tile([PT, C], I32)
            nc.vector.tensor_scalar(rmod_i[:], idxT_i[:], 15, None, op0=alu.bitwise_and)
            rmod = sb.tile([PT, C], F32)
            nc.vector.tensor_copy(rmod[:], rmod_i[:])
            mask = sb.tile([PT, C], F32)
            nc.vector.tensor_scalar(mask[:], rmod[:], p16f[:], None, op0=alu.is_equal)
            col1 = sb.tile([PT, C], F32)  # idxT // 16 + 1
            nc.vector.tensor_tensor(col1[:], idxT[:], rmod[:], op=alu.subtract)
            nc.vector.tensor_scalar(col1[:], col1[:], 1.0 / 16.0, 1.0, op0=alu.mult, op1=alu.add)
            sc_idx_f = sb.tile([PT, C], F32)  # mask*col1 - 1 (-1 = ignore)
            nc.vector.tensor_tensor(sc_idx_f[:], mask[:], col1[:], op=alu.mult)
            nc.vector.tensor_scalar(sc_idx_f[:], sc_idx_f[:], 1.0, None, op0=alu.subtract)
            sc_idx = sb.tile([PT, C], mybir.dt.int16)
            nc.vector.tensor_copy(sc_idx[:], sc_idx_f[:])

            inv_w = sb.tile([PT, P // 16], U16)
            nc.gpsimd.local_scatter(
                out_ap=inv_w[:],
                data_ap=iota_c1[:],
                idxs_ap=sc_idx[:],
                channels=PT,
                num_elems=P // 16,
                num_idxs=C,
            )

            # ---- outT[d, p] = combT[d, inv[p]]; transpose back; store ----
            outN = sb.tile([PT, NDH, P // PT, PT], F32, tag="big", bufs=1)
            for h in range(NDH):
                outT = sbs.tile([PT, P], F32, tag="outT", bufs=1)
                for k in range(4):  # ISA: <=1024 dst elems per indirect_copy
                    nc.gpsimd.indirect_copy(
                        outT[:, 1024 * k:1024 * (k + 1)],
                        data=combT[:, h, :],
                        idxs=inv_w[:, 64 * k:64 * (k + 1)],
                        i_know_ap_gather_is_preferred=True,
                    )
                for g in range(8):
                    pot = pp.tile([PT, 512], F32, tag="pt")
                    for s in range(4):
                        t = 4 * g + s
                        nc.tensor.transpose(
                            pot[:, 128 * s:128 * (s + 1)],
                            outT[:, PT * t:PT * (t + 1)],
                            ident[:],
                        )
                    nc.scalar.copy(
                        outN[:, h, 4 * g:4 * (g + 1), :],
                        pot[:].rearrange("p (s dd) -> p s dd", dd=PT),
                    )
            o_view = o_d[:].rearrange("(q t p) (h dd) -> h q p t dd", q=4, p=PT, dd=PT)
            for h in range(NDH):
                for q in range(4):
                    nc.sync.dma_start(o_view[h][q], outN[:, h, 8 * q:8 * (q + 1), :])

    nc.compile()
    return nc


def _get_nc():
    if "nc" not in _NC_CACHE:
        _NC_CACHE["nc"] = _build_nc()
    return _NC_CACHE["nc"]


def kernel(x: np.ndarray, W: np.ndarray) -> np.ndarray:
    x = np.ascontiguousarray(x, dtype=np.float32)
    W = np.ascontiguousarray(W, dtype=np.float32)
    assert x.shape == (B, P, D) and W.shape == (C, D)
    nc = _get_nc()
    in_maps = [{"x": x[b], "w": W} for b in range(B)]
    res = run_bass_kernel_spmd(nc, in_maps, core_ids=list(range(B)))
    out = np.stack([res.results[b]["o"] for b in range(B)], axis=0)
    return out.astype(np.float32)


if __name__ == "__main__":
    rng = np.random.default_rng(0)
    x = rng.standard_normal((B, P, D), dtype=np.float32)
    W = (rng.standard_normal((C, D), dtype=np.float32) * 0.001).astype(np.float32)
    out = kernel(x=x, W=W)
    print(out.shape, out.dtype, float(np.abs(out).sum()))
